# revision 13
# baseline (speedup 1.0000x reference)
"""PolyGAN CP layer kernel for 8 trn2 NeuronCores.

Math (N=5, RANK=4, S=1024*1024):
    d[k-1, r] = dot(z, W[k][:, r])   k = 1..3          -> 12 scalars
    coef      = 2 + sum(cumprod(d, axis=0), axis=0)    -> 4 scalars
    out       = W[0] @ coef + b                        -> (S,)

Only W[0:4] of the 20 factor matrices is used.

Sharding: s-dimension split across 8 cores (131072 rows each). Each core
computes partial dot products of its shard (12 scalars), an AllGather shares
the per-core partials, every core reduces them + computes coef locally, then
produces its shard of W[0] @ coef + b.

Perf notes vs the first working version:
 - inputs cast to bf16 on the host (rel-err budget 2e-2, bf16 lands ~4e-3),
   halving HBM traffic and doubling DVE throughput;
 - the 12 per-plane free-dim reductions are split between ScalarE (activation
   accumulator) and DVE (tensor_reduce) so neither engine paces the pipeline;
 - all-DVE bf16 phase 2, gather-back on the idle sync HWDGE ring, split
   final add + store, and no partition-id preamble, trimming the
   post-collective tail (the ncfw collective engine only becomes available
   ~65-75 us into an execution, so everything before the AllGather hides
   under that floor and the tail is what matters).
"""

import sys

for _p in ("/opt/trn_rl_repo",):
    if _p not in sys.path:
        sys.path.insert(0, _p)

import numpy as np

S = 1048576
N_CORES = 8
SH = S // N_CORES  # 131072 per core
P = 128
F = SH // P  # 1024

_CACHE = {}


def _build_bass():
    import concourse.bacc as bacc
    import concourse.mybir as mybir
    import concourse.tile as tile

    f32 = mybir.dt.float32
    bf16 = mybir.dt.bfloat16
    Alu = mybir.AluOpType
    Act = mybir.ActivationFunctionType

    nc = bacc.Bacc("TRN2", target_bir_lowering=False, debug=False,
                   num_devices=N_CORES, enable_partition_id=False)

    z_d = nc.dram_tensor("z", [P, F], bf16, kind="ExternalInput")
    wk_d = nc.dram_tensor("wk", [12, P, F], bf16, kind="ExternalInput")
    w0_d = nc.dram_tensor("w0", [4, P, F], bf16, kind="ExternalInput")
    b_d = nc.dram_tensor("b", [P, F], bf16, kind="ExternalInput")
    out_d = nc.dram_tensor("out", [P, F], f32, kind="ExternalOutput")

    with tile.TileContext(nc) as tc:
        with tc.tile_pool(name="sbuf", bufs=1) as pool, \
             tc.tile_pool(name="scr", bufs=2) as scrpool, \
             tc.tile_pool(name="psum", bufs=1, space="PSUM") as psum, \
             tc.tile_pool(name="dram", bufs=1, space="DRAM") as dram:

            # ---- phase 1 loads: z + the 12 (k, r) planes of W[1:4] ----
            with nc.named_scope("load_issue"):
                z_t = pool.tile([P, F], bf16)
                nc.sync.dma_start(z_t[:], z_d.ap())
                wk_t = []
                for j in range(12):
                    t = pool.tile([P, F], bf16, tag=f"wk{j}")
                    nc.sync.dma_start(t[:], wk_d.ap()[j])
                    wk_t.append(t)

                # ---- phase 2 loads (independent of phase 1 compute) ----
                w0_t = []
                for r in range(4):
                    t = pool.tile([P, F], bf16, tag=f"w0{r}")
                    nc.sync.dma_start(t[:], w0_d.ap()[r])
                    w0_t.append(t)
                b_t = pool.tile([P, F], bf16)
                nc.sync.dma_start(b_t[:], b_d.ap())

            # ---- phase 1: per-plane dot -> partials[128, 12] ----
            # DVE multiplies; the free-dim row-sum alternates between the
            # ScalarE activation accumulator and DVE tensor_reduce so the
            # reduce never paces the (DMA-bound) pipeline.
            with nc.named_scope("dots"):
                partials = pool.tile([P, 12], f32)
                for j in range(12):
                    scr = scrpool.tile([P, F], bf16, tag="scr")
                    nc.vector.tensor_tensor(scr[:], wk_t[j][:], z_t[:],
                                            Alu.mult)
                    if j % 2 == 0:
                        red = scrpool.tile([P, F], bf16, tag="red",
                                           name="red")
                        nc.scalar.activation(
                            red[:], scr[:], Act.Copy,
                            accum_out=partials[:, j:j + 1])
                    else:
                        nc.vector.tensor_reduce(
                            partials[:, j:j + 1], scr[:],
                            axis=mybir.AxisListType.X, op=Alu.add)

                # cross-partition reduce: ones[128,1].T @ partials -> [1, 12]
                # (ones/zeros derived from z so no op precedes the DMAs)
                ones_col = pool.tile([P, 1], f32)
                nc.vector.tensor_scalar(ones_col[:], z_t[:, 0:1], 0.0, 1.0,
                                        Alu.mult, Alu.add)
                d_ps = psum.tile([1, 12], f32)
                nc.tensor.matmul(d_ps[:], lhsT=ones_col[:], rhs=partials[:],
                                 start=True, stop=True)
                d_sb = pool.tile([1, 16], f32)
                nc.vector.tensor_scalar(d_sb[:], z_t[0:1, 0:16], 0.0, None,
                                        Alu.mult)
                nc.vector.tensor_copy(d_sb[:, 0:12], d_ps[:])

            # ---- share partials across cores ----
            with nc.named_scope("exchange"):
                cc_in = dram.tile([1, 16], f32)
                cc_out = dram.tile([N_CORES, 16], f32)
                nc.sync.dma_start(cc_in[:], d_sb[:])
                nc.gpsimd.collective_compute(
                    "AllGather",
                    Alu.bypass,
                    replica_groups=[list(range(N_CORES))],
                    ins=[cc_in.opt()],
                    outs=[cc_out.opt()],
                )
                gath = pool.tile([1, N_CORES * 16], f32)
                nc.sync.dma_start(gath[:],
                                  cc_out[:].rearrange("a b -> (a b)"))

            sc_tail = nc.named_scope("tail")
            sc_tail.__enter__()
            # sum the 8 per-core partial vectors -> d_full [1, 16]
            d_full = pool.tile([1, 16], f32)
            nc.vector.tensor_reduce(
                d_full[:],
                gath[:].rearrange("p (g j) -> p j g", g=N_CORES),
                axis=mybir.AxisListType.X,
                op=Alu.add,
            )

            # ---- coef = 2 + d1 + d1*d2 + d1*d2*d3 (elementwise over r) ----
            d1 = d_full[:, 0:4]
            d2 = d_full[:, 4:8]
            d3 = d_full[:, 8:12]
            t1 = pool.tile([1, 4], f32)
            nc.vector.tensor_scalar_add(t1[:], d1, 2.0)
            cum2 = pool.tile([1, 4], f32)
            nc.vector.tensor_tensor(cum2[:], d1, d2, Alu.mult)
            cum3 = pool.tile([1, 4], f32)
            nc.vector.tensor_tensor(cum3[:], cum2[:], d3, Alu.mult)
            t2 = pool.tile([1, 4], f32)
            nc.vector.tensor_tensor(t2[:], t1[:], cum2[:], Alu.add)
            coef = pool.tile([1, 4], f32)
            nc.vector.tensor_tensor(coef[:], t2[:], cum3[:], Alu.add)

            # broadcast coef across partitions: ones[1,128].T @ coef[1,4]
            ones_row = pool.tile([1, P], f32)
            nc.vector.tensor_scalar(ones_row[:], z_t[0:1, 0:P], 0.0, 1.0,
                                    Alu.mult, Alu.add)
            cb_ps = psum.tile([P, 4], f32)
            nc.tensor.matmul(cb_ps[:], lhsT=ones_row[:], rhs=coef[:],
                             start=True, stop=True)
            coefb = cb_ps

            # ---- phase 2: out = sum_r coef_r * W0plane_r + b ----
            m0 = pool.tile([P, F], bf16)
            nc.vector.tensor_scalar(m0[:], w0_t[0][:], coefb[:, 0:1], None,
                                    Alu.mult)
            m1 = pool.tile([P, F], bf16)
            nc.vector.tensor_scalar(m1[:], w0_t[1][:], coefb[:, 1:2], None,
                                    Alu.mult)
            m2 = pool.tile([P, F], bf16)
            nc.vector.tensor_scalar(m2[:], w0_t[2][:], coefb[:, 2:3], None,
                                    Alu.mult)
            m3 = pool.tile([P, F], bf16)
            nc.vector.tensor_scalar(m3[:], w0_t[3][:], coefb[:, 3:4], None,
                                    Alu.mult)
            s01 = pool.tile([P, F], bf16)
            nc.vector.tensor_tensor(s01[:], m0[:], m1[:], Alu.add)
            s23 = pool.tile([P, F], bf16)
            nc.vector.tensor_tensor(s23[:], m2[:], m3[:], Alu.add)
            s03 = pool.tile([P, F], bf16)
            nc.vector.tensor_tensor(s03[:], s01[:], s23[:], Alu.add)
            res = pool.tile([P, F], f32)
            H = F // 2
            nc.vector.tensor_tensor(res[:, 0:H], s03[:, 0:H], b_t[:, 0:H],
                                    Alu.add)
            nc.sync.dma_start(out_d.ap()[:, 0:H], res[:, 0:H])
            nc.vector.tensor_tensor(res[:, H:F], s03[:, H:F], b_t[:, H:F],
                                    Alu.add)
            nc.sync.dma_start(out_d.ap()[:, H:F], res[:, H:F])
            sc_tail.__exit__(None, None, None)

    nc.compile()
    return nc


def _get_nc():
    if "nc" not in _CACHE:
        _CACHE["nc"] = _build_bass()
    return _CACHE["nc"]


def _make_in_maps(z, W, b):
    import ml_dtypes

    bf = ml_dtypes.bfloat16
    z = np.asarray(z, dtype=np.float32)
    W = np.asarray(W, dtype=np.float32)
    b = np.asarray(b, dtype=np.float32)

    in_maps = []
    for c in range(N_CORES):
        sl = slice(c * SH, (c + 1) * SH)
        wk_c = np.ascontiguousarray(
            W[1:4, sl, :].transpose(0, 2, 1)).reshape(12, P, F).astype(bf)
        w0_c = np.ascontiguousarray(W[0, sl, :].T).reshape(4, P, F).astype(bf)
        in_maps.append({
            "z": np.ascontiguousarray(z[sl]).reshape(P, F).astype(bf),
            "wk": wk_c,
            "w0": w0_c,
            "b": np.ascontiguousarray(b[sl]).reshape(P, F).astype(bf),
        })
    return in_maps


def kernel(z, W, b):
    from concourse.bass_utils import run_bass_kernel_spmd

    nc = _get_nc()
    in_maps = _make_in_maps(z, W, b)
    res = run_bass_kernel_spmd(nc, in_maps, core_ids=list(range(N_CORES)),
                               trace=False)
    return np.concatenate(
        [res.results[c]["out"].reshape(-1) for c in range(N_CORES)])


# revision 15
# speedup vs baseline: 1.1616x; 1.1616x over previous
"""PolyGAN CP layer kernel for 8 trn2 NeuronCores.

Math (N=5, RANK=4, S=1024*1024):
    d[k-1, r] = dot(z, W[k][:, r])   k = 1..3          -> 12 scalars
    coef      = 2 + sum(cumprod(d, axis=0), axis=0)    -> 4 scalars
    out       = W[0] @ coef + b                        -> (S,)

Only W[0:4] of the 20 factor matrices is used.

Sharding: s-dimension split across 8 cores (131072 rows each). Each core
computes partial dot products of its shard (12 scalars), an AllGather shares
the per-core partials, every core reduces them + computes coef locally, then
produces its shard of W[0] @ coef + b.

Perf notes vs the first working version:
 - inputs cast to bf16 on the host (rel-err budget 2e-2, bf16 lands ~4e-3),
   halving HBM traffic and doubling DVE throughput;
 - the 12 per-plane free-dim reductions are split between ScalarE (activation
   accumulator) and DVE (tensor_reduce) so neither engine paces the pipeline;
 - all-DVE bf16 phase 2, gather-back on the idle sync HWDGE ring, split
   final add + store, and no partition-id preamble, trimming the
   post-collective tail (the ncfw collective engine only becomes available
   ~65-75 us into an execution, so everything before the AllGather hides
   under that floor and the tail is what matters).
"""

import sys

for _p in ("/opt/trn_rl_repo",):
    if _p not in sys.path:
        sys.path.insert(0, _p)

import numpy as np

S = 1048576
N_CORES = 8
SH = S // N_CORES  # 131072 per core
P = 128
F = SH // P  # 1024

_CACHE = {}


def _build_bass():
    import concourse.bacc as bacc
    import concourse.mybir as mybir
    import concourse.tile as tile

    f32 = mybir.dt.float32
    bf16 = mybir.dt.bfloat16
    Alu = mybir.AluOpType
    Act = mybir.ActivationFunctionType

    nc = bacc.Bacc("TRN2", target_bir_lowering=False, debug=False,
                   num_devices=N_CORES, enable_partition_id=False)

    z_d = nc.dram_tensor("z", [P, F], bf16, kind="ExternalInput")
    wk_d = nc.dram_tensor("wk", [12, P, F], bf16, kind="ExternalInput")
    w0_d = nc.dram_tensor("w0", [4, P, F], bf16, kind="ExternalInput")
    b_d = nc.dram_tensor("b", [P, F], bf16, kind="ExternalInput")
    out_d = nc.dram_tensor("out", [P, F], f32, kind="ExternalOutput")

    with tile.TileContext(nc) as tc:
        with tc.tile_pool(name="sbuf", bufs=1) as pool, \
             tc.tile_pool(name="scr", bufs=2) as scrpool, \
             tc.tile_pool(name="psum", bufs=1, space="PSUM") as psum, \
             tc.tile_pool(name="dram", bufs=1, space="DRAM") as dram:

            # ---- phase 1 loads: z + the 12 (k, r) planes of W[1:4] ----
            with nc.named_scope("load_issue"):
                z_t = pool.tile([P, F], bf16)
                nc.sync.dma_start(z_t[:], z_d.ap())
                wk_t = []
                for j in range(12):
                    t = pool.tile([P, F], bf16, tag=f"wk{j}")
                    nc.sync.dma_start(t[:], wk_d.ap()[j])
                    wk_t.append(t)

                # ---- phase 2 loads (independent of phase 1 compute) ----
                w0_t = []
                for r in range(4):
                    t = pool.tile([P, F], bf16, tag=f"w0{r}")
                    nc.sync.dma_start(t[:], w0_d.ap()[r])
                    w0_t.append(t)
                b_t = pool.tile([P, F], bf16)
                nc.sync.dma_start(b_t[:], b_d.ap())

            # ---- phase 1: per-plane dot -> partials[128, 12] ----
            # DVE multiplies; the free-dim row-sum alternates between the
            # ScalarE activation accumulator and DVE tensor_reduce so the
            # reduce never paces the (DMA-bound) pipeline.
            with nc.named_scope("dots"):
                partials = pool.tile([P, 12], f32)
                for j in range(12):
                    scr = scrpool.tile([P, F], bf16, tag="scr")
                    nc.vector.tensor_tensor(scr[:], wk_t[j][:], z_t[:],
                                            Alu.mult)
                    if j % 2 == 0:
                        red = scrpool.tile([P, F], bf16, tag="red",
                                           name="red")
                        nc.scalar.activation(
                            red[:], scr[:], Act.Copy,
                            accum_out=partials[:, j:j + 1])
                    else:
                        nc.vector.tensor_reduce(
                            partials[:, j:j + 1], scr[:],
                            axis=mybir.AxisListType.X, op=Alu.add)

                # cross-partition reduce: ones[128,1].T @ partials -> [1, 12]
                # (ones/zeros derived from z so no op precedes the DMAs)
                ones_col = pool.tile([P, 1], f32)
                nc.vector.tensor_scalar(ones_col[:], z_t[:, 0:1], 0.0, 1.0,
                                        Alu.mult, Alu.add)
                d_ps = psum.tile([1, 12], f32)
                nc.tensor.matmul(d_ps[:], lhsT=ones_col[:], rhs=partials[:],
                                 start=True, stop=True)
                d_sb = pool.tile([1, 16], f32)
                nc.vector.tensor_scalar(d_sb[:], z_t[0:1, 0:16], 0.0, None,
                                        Alu.mult)
                nc.vector.tensor_copy(d_sb[:, 0:12], d_ps[:])

            # ---- share partials across cores ----
            with nc.named_scope("exchange"):
                cc_in = dram.tile([1, 16], f32)
                cc_out = dram.tile([N_CORES, 16], f32)
                nc.sync.dma_start(cc_in[:], d_sb[:])
                nc.gpsimd.collective_compute(
                    "AllGather",
                    Alu.bypass,
                    replica_groups=[list(range(N_CORES))],
                    ins=[cc_in.opt()],
                    outs=[cc_out.opt()],
                )
                gath = pool.tile([1, N_CORES * 16], f32)
                nc.sync.dma_start(gath[:],
                                  cc_out[:].rearrange("a b -> (a b)"))

            sc_tail = nc.named_scope("tail")
            sc_tail.__enter__()
            # sum the 8 per-core partial vectors -> d_full [1, 16]
            d_full = pool.tile([1, 16], f32)
            nc.vector.tensor_reduce(
                d_full[:],
                gath[:].rearrange("p (g j) -> p j g", g=N_CORES),
                axis=mybir.AxisListType.X,
                op=Alu.add,
            )

            # ---- coef = 2 + d1 + d1*d2 + d1*d2*d3 (elementwise over r) ----
            d1 = d_full[:, 0:4]
            d2 = d_full[:, 4:8]
            d3 = d_full[:, 8:12]
            t1 = pool.tile([1, 4], f32)
            nc.vector.tensor_scalar_add(t1[:], d1, 2.0)
            cum2 = pool.tile([1, 4], f32)
            nc.vector.tensor_tensor(cum2[:], d1, d2, Alu.mult)
            cum3 = pool.tile([1, 4], f32)
            nc.vector.tensor_tensor(cum3[:], cum2[:], d3, Alu.mult)
            t2 = pool.tile([1, 4], f32)
            nc.vector.tensor_tensor(t2[:], t1[:], cum2[:], Alu.add)
            coef = pool.tile([1, 4], f32)
            nc.vector.tensor_tensor(coef[:], t2[:], cum3[:], Alu.add)

            # broadcast coef across partitions: ones[1,128].T @ coef[1,4]
            ones_row = pool.tile([1, P], f32)
            nc.vector.tensor_scalar(ones_row[:], z_t[0:1, 0:P], 0.0, 1.0,
                                    Alu.mult, Alu.add)
            cb_ps = psum.tile([P, 4], f32)
            nc.tensor.matmul(cb_ps[:], lhsT=ones_row[:], rhs=coef[:],
                             start=True, stop=True)
            coefb = cb_ps

            # ---- phase 2: out = sum_r coef_r * W0plane_r + b ----
            m0 = pool.tile([P, F], bf16)
            nc.vector.tensor_scalar(m0[:], w0_t[0][:], coefb[:, 0:1], None,
                                    Alu.mult)
            m1 = pool.tile([P, F], bf16)
            nc.vector.tensor_scalar(m1[:], w0_t[1][:], coefb[:, 1:2], None,
                                    Alu.mult)
            m2 = pool.tile([P, F], bf16)
            nc.vector.tensor_scalar(m2[:], w0_t[2][:], coefb[:, 2:3], None,
                                    Alu.mult)
            m3 = pool.tile([P, F], bf16)
            nc.vector.tensor_scalar(m3[:], w0_t[3][:], coefb[:, 3:4], None,
                                    Alu.mult)
            s01 = pool.tile([P, F], bf16)
            nc.vector.tensor_tensor(s01[:], m0[:], m1[:], Alu.add)
            s23 = pool.tile([P, F], bf16)
            nc.vector.tensor_tensor(s23[:], m2[:], m3[:], Alu.add)
            s03 = pool.tile([P, F], bf16)
            nc.vector.tensor_tensor(s03[:], s01[:], s23[:], Alu.add)
            res = pool.tile([P, F], f32)
            H = F // 2
            nc.vector.tensor_tensor(res[:, 0:H], s03[:, 0:H], b_t[:, 0:H],
                                    Alu.add)
            nc.sync.dma_start(out_d.ap()[:, 0:H], res[:, 0:H])
            nc.vector.tensor_tensor(res[:, H:F], s03[:, H:F], b_t[:, H:F],
                                    Alu.add)
            nc.sync.dma_start(out_d.ap()[:, H:F], res[:, H:F])
            sc_tail.__exit__(None, None, None)

    nc.compile()
    return nc


def _get_nc():
    if "nc" not in _CACHE:
        _CACHE["nc"] = _build_bass()
    return _CACHE["nc"]


def _make_in_maps(z, W, b):
    import ml_dtypes

    bf = ml_dtypes.bfloat16
    z = np.asarray(z, dtype=np.float32)
    W = np.asarray(W, dtype=np.float32)
    b = np.asarray(b, dtype=np.float32)

    in_maps = []
    for c in range(N_CORES):
        sl = slice(c * SH, (c + 1) * SH)
        wk_c = np.ascontiguousarray(
            W[1:4, sl, :].transpose(0, 2, 1)).reshape(12, P, F).astype(bf)
        w0_c = np.ascontiguousarray(W[0, sl, :].T).reshape(4, P, F).astype(bf)
        in_maps.append({
            "z": np.ascontiguousarray(z[sl]).reshape(P, F).astype(bf),
            "wk": wk_c,
            "w0": w0_c,
            "b": np.ascontiguousarray(b[sl]).reshape(P, F).astype(bf),
        })
    return in_maps


def kernel(z, W, b):
    from concourse.bass_utils import run_bass_kernel_spmd

    nc = _get_nc()
    in_maps = _make_in_maps(z, W, b)
    res = run_bass_kernel_spmd(nc, in_maps, core_ids=list(range(N_CORES)),
                               trace=False)
    return np.concatenate(
        [res.results[c]["out"].reshape(-1) for c in range(N_CORES)])
